# revision 20
# baseline (speedup 1.0000x reference)
"""Bahdanau additive-attention kernel for Trainium2, SPMD over 8 NeuronCores.

Reference computation (per batch b):
    dec_t  = dec @ W                                  [TD, D]
    score  = sum_d V[d] * tanh(dec_t[td,d] + enc[te,d])   [TD, TE]
    attn   = softmax(score, axis=te)
    ctx    = attn @ enc                               [TD, D]

Sharding: B=4, 8 cores -> core c handles batch b = c//2, td half h = c%2
(256 td rows each); enc/W replicated per batch. Host side does layout
marshalling only; all FLOPs of the reference computation run on device.

Algorithm: instead of evaluating tanh over the full [TD, TE, D] volume
(16.8M ACT elems/core ~ 110 us), use a Fourier-feature factorization:
    tanh(s) ~ sum_m b_m sin(w_m s),   w_m = (m-1/2)*pi/L,  m=1..K
and since sin(w(x+y)) = sin(wx)cos(wy) + cos(wx)sin(wy), the score
becomes a plain GEMM over an expanded inner dim D*2K:
    score[td,te] = sum_{m,d} V_d b_m [sin_m(a)cos_m(e) + cos_m(a)sin_m(e)]
with a = dec_t, e = enc. Feature work is only (TD_N + TE)*D*2K elems.

HW Sin is valid on [-pi, pi] only and the ISA has no mod op, so phases
are range-reduced with the fp32 magic-number rounding trick (the DVE/Pool
ALU computes tensor_scalar chains internally in fp32):
    q   = x * (w_m / 2pi)                        (tensor_scalar mult)
    rq  = (q + 1.5*2^23) - 1.5*2^23  = round(q)  (tensor_scalar add,sub)
    d   = q - rq              in [-1/2, 1/2]     (tensor_tensor sub)
    mask= (d > 1/4) ? 1 : 0                      (tensor_scalar is_gt)
    d2  = (d + 1/4) - mask    in [-1/2, 1/2]     (scalar_tensor_tensor)
then ACT computes Sin(d * 2pi) = sin(w x), Sin(d2 * 2pi) = cos(w x),
both halves in one batched Sin instruction (scale=2pi, bias=0).
rq/rq2/d/d2 are batched across the modes of a pipeline batch (their
scalars are mode-independent). The a-side runs on Pool (1x, but small
tiles); e-side + STT (DVE-only) + coefficient folds run on DVE.
Coefficients c[d,m] = V_d*b_m are folded into the (smaller) a-side
features with batched tensor_tensor multiplies.

Score is accumulated TRANSPOSED, scoreT[te, td], via lhsT=G (e-side)
rhs=F (a-side) matmuls, so the softmax epilogue needs no PE transposes:
exp -> escT is already [te, td] = lhsT for the context matmul against
[enc | 1] (denominator in the extra column), then reciprocal * scale.
"""

from contextlib import ExitStack

import numpy as np

import concourse.bacc as bacc
import concourse.tile as tile
from concourse import mybir
from concourse.bass_utils import run_bass_kernel_spmd

F32 = mybir.dt.float32
F16 = mybir.dt.float16
BF16 = mybir.dt.bfloat16
ALU = mybir.AluOpType

B, TD, TE, D = 4, 512, 512, 128
N_CORES = 8
TD_N = (B * TD) // N_CORES          # 256 td rows per core
P = 128
N_CHUNK = TE // P                   # 4 te chunks

# --- Fourier fit of tanh on [-L, L], half-integer sine modes ---
K_MODES = 12
FIT_L = 10.5


def _fit_tanh_modes(L=FIT_L, K=K_MODES):
    om = (np.arange(1, K + 1) - 0.5) * np.pi / L
    s = np.linspace(-L, L, 8001)
    w = np.exp(-0.5 * (s / 1.9) ** 2) + 1e-3 * (np.abs(s) < L - 0.7)
    M = np.sin(np.outer(s, om))
    Wt = np.sqrt(w)[:, None]
    b, *_ = np.linalg.lstsq(M * Wt, np.tanh(s) * Wt[:, 0], rcond=None)
    return om.astype(np.float64), b.astype(np.float64)

OMEGA, BCOEF = _fit_tanh_modes()

TWO_PI = float(2 * np.pi)
MAGIC = float(1.5 * 2**23)          # fp32 round-to-nearest-integer bias

# modes per pipeline batch (sum == K_MODES); small tail batches shrink the
# PE drain after the last ACT feature instruction
BATCHES = [4, 4, 4]

DEBUG_SCORE = None  # set to a [dram_ap] list in debug builds


def _build_body(ctx, tc, out_ap, decT_ap, encT_ap, enc_ones_ap, coef_ap, w_ap):
    nc = tc.nc

    consts = ctx.enter_context(tc.tile_pool(name="consts", bufs=1))
    setup_ps = ctx.enter_context(tc.tile_pool(name="setup_ps", bufs=1, space="PSUM"))
    st_ps_pool = ctx.enter_context(tc.tile_pool(name="st_ps", bufs=1, space="PSUM"))
    ctx_ps_pool = ctx.enter_context(tc.tile_pool(name="ctx_ps", bufs=1, space="PSUM"))
    qe_pool = ctx.enter_context(tc.tile_pool(name="qe", bufs=2))
    rq_pool = ctx.enter_context(tc.tile_pool(name="rq", bufs=2))
    ve_pool = ctx.enter_context(tc.tile_pool(name="ve", bufs=2))
    qa_pool = ctx.enter_context(tc.tile_pool(name="qa", bufs=2))
    rqa_pool = ctx.enter_context(tc.tile_pool(name="rqa", bufs=2))
    va_pool = ctx.enter_context(tc.tile_pool(name="va", bufs=2))
    ge_pool = ctx.enter_context(tc.tile_pool(name="ge", bufs=2))
    fa_pool = ctx.enter_context(tc.tile_pool(name="fa", bufs=2))
    fs_pool = ctx.enter_context(tc.tile_pool(name="fs", bufs=2))
    esc_pool = ctx.enter_context(tc.tile_pool(name="esc", bufs=4))
    out_pool = ctx.enter_context(tc.tile_pool(name="outp", bufs=2))

    # ---- input DMAs ----
    encT = consts.tile([P, TE], F16)              # [d, te] fp16 phases src
    nc.sync.dma_start(out=encT, in_=encT_ap)
    decT = consts.tile([P, TD_N], F32)            # [d, td] fp32
    nc.scalar.dma_start(out=decT, in_=decT_ap)
    w_sb = consts.tile([P, P], F32)
    nc.scalar.dma_start(out=w_sb, in_=w_ap)
    coef = consts.tile([P, K_MODES, TD_N], BF16)  # V_d*b_m bcast over td
    nc.gpsimd.dma_start(out=coef, in_=coef_ap)
    enc_ones = consts.tile([P, N_CHUNK, P + 1], BF16)   # [te | 1.0]
    nc.gpsimd.dma_start(out=enc_ones, in_=enc_ones_ap)

    # ---- dec_t = (dec @ W)^T on PE (fp32 for phase accuracy) ----
    dec_t_ps = setup_ps.tile([P, TD_N], F32)
    nc.tensor.matmul(dec_t_ps, w_sb, decT, start=True, stop=True)
    dec_tT = consts.tile([P, TD_N], F32)
    nc.vector.tensor_copy(dec_tT, dec_t_ps)

    # scoreT accumulator: [te(4x128), td] fp32. Each te chunk gets its own
    # 2KB PSUM bank (start_tensor_calc zeroes the whole bank, so concurrent
    # accumulation groups must not share one); cols TD_N..511 are padding.
    st_full = st_ps_pool.tile([P, N_CHUNK, 512], F32)
    st = st_full[:, :, 0:TD_N]

    n_mm = 2 * K_MODES  # accumulating matmuls per chunk
    mm_i = 0
    m0 = 0
    for bs in BATCHES:
        # e-side phase args on DVE (fp16 4x mode)
        qe = qe_pool.tile([P, bs, TE], F16, tag="qe")
        for j in range(bs):
            sc = float(OMEGA[m0 + j] / (2 * np.pi))
            nc.vector.tensor_scalar(
                out=qe[:, j, :], in0=encT, scalar1=sc, scalar2=None, op0=ALU.mult
            )
        rqe = rq_pool.tile([P, 2, bs, TE], F16, tag="rqe")
        nc.vector.tensor_scalar(
            out=rqe[:, 0], in0=qe, scalar1=MAGIC, scalar2=MAGIC,
            op0=ALU.add, op1=ALU.subtract,
        )
        ve = ve_pool.tile([P, 2, bs, TE], F16, tag="ve")
        nc.vector.tensor_tensor(out=ve[:, 0], in0=qe, in1=rqe[:, 0], op=ALU.subtract)
        nc.vector.tensor_scalar(
            out=rqe[:, 1], in0=ve[:, 0], scalar1=0.25, scalar2=None, op0=ALU.is_gt
        )
        nc.vector.scalar_tensor_tensor(
            out=ve[:, 1], in0=ve[:, 0], scalar=0.25, in1=rqe[:, 1],
            op0=ALU.add, op1=ALU.subtract,
        )

        # a-side phase args on Pool (except STT, DVE-only)
        qa = qa_pool.tile([P, bs, TD_N], F16, tag="qa")
        for j in range(bs):
            sc = float(OMEGA[m0 + j] / (2 * np.pi))
            nc.gpsimd.tensor_scalar(
                out=qa[:, j, :], in0=dec_tT, scalar1=sc, scalar2=None, op0=ALU.mult
            )
        rqa = rqa_pool.tile([P, 2, bs, TD_N], F16, tag="rqa")
        nc.gpsimd.tensor_scalar(
            out=rqa[:, 0], in0=qa, scalar1=MAGIC, scalar2=MAGIC,
            op0=ALU.add, op1=ALU.subtract,
        )
        va = va_pool.tile([P, 2, bs, TD_N], F16, tag="va")
        nc.gpsimd.tensor_tensor(out=va[:, 0], in0=qa, in1=rqa[:, 0], op=ALU.subtract)
        nc.gpsimd.tensor_scalar(
            out=rqa[:, 1], in0=va[:, 0], scalar1=0.25, scalar2=None, op0=ALU.is_gt
        )
        nc.vector.scalar_tensor_tensor(
            out=va[:, 1], in0=va[:, 0], scalar=0.25, in1=rqa[:, 1],
            op0=ALU.add, op1=ALU.subtract,
        )

        # features: ge[:,0,j,:] = sin(w e), ge[:,1,j,:] = cos(w e); same a-side
        ge = ge_pool.tile([P, 2, bs, TE], BF16, tag="ge")
        nc.scalar.activation(
            out=ge, in_=ve, func=mybir.ActivationFunctionType.Sin, scale=TWO_PI
        )
        fa = fa_pool.tile([P, 2, bs, TD_N], BF16, tag="fa")
        nc.scalar.activation(
            out=fa, in_=va, func=mybir.ActivationFunctionType.Sin, scale=TWO_PI
        )

        # fold c[d,m] = V_d*b_m into a-side features (both trig halves), Pool
        fsc = fs_pool.tile([P, 2, bs, TD_N], BF16, tag="fsc")
        nc.gpsimd.tensor_tensor(
            out=fsc[:, 0, :, :], in0=fa[:, 0, :, :],
            in1=coef[:, m0 : m0 + bs, :], op=ALU.mult,
        )
        nc.gpsimd.tensor_tensor(
            out=fsc[:, 1, :, :], in0=fa[:, 1, :, :],
            in1=coef[:, m0 : m0 + bs, :], op=ALU.mult,
        )

        # scoreT += G_sin^T F~_cos + G_cos^T F~_sin  (sign flips cancel)
        for j in range(bs):
            for half in (0, 1):
                for c in range(N_CHUNK):
                    nc.tensor.matmul(
                        st[:, c, :],
                        ge[:, half, j, c * P : (c + 1) * P],
                        fsc[:, 1 - half, j, :],
                        start=(mm_i == 0),
                        stop=(mm_i == n_mm - 1),
                        skip_group_check=True,
                    )
                mm_i += 1
        m0 += bs

    if DEBUG_SCORE is not None:
        dbg = consts.tile([P, N_CHUNK, TD_N], F32)
        nc.vector.tensor_copy(dbg, st)
        nc.sync.dma_start(out=DEBUG_SCORE[0], in_=dbg)

    # ---- softmax + context, chunk-staggered ----
    # one full PSUM bank per block: the two blocks' accumulation groups
    # interleave, so they must not share a bank
    ctx_ps = [
        ctx_ps_pool.tile([P, 512], F32, tag=f"ctx{b}", name=f"ctx_ps{b}")[:, 0 : P + 1]
        for b in range(2)
    ]
    for c in range(N_CHUNK):
        escT = esc_pool.tile([P, TD_N], BF16, tag=f"escT{c}")
        nc.scalar.activation(
            out=escT, in_=st[:, c, :], func=mybir.ActivationFunctionType.Exp
        )
        for blk in range(2):
            nc.tensor.matmul(
                ctx_ps[blk], escT[:, blk * P : (blk + 1) * P], enc_ones[:, c, :],
                start=(c == 0), stop=(c == N_CHUNK - 1),
            )
    dma_engs = [nc.sync, nc.scalar]
    for blk in range(2):
        recip = out_pool.tile([P, 1], F32, tag=f"recip{blk}")
        nc.vector.reciprocal(recip, ctx_ps[blk][:, P : P + 1])
        ctx_sb = out_pool.tile([P, P], F32, tag=f"ctx_sb{blk}")
        nc.vector.tensor_scalar_mul(out=ctx_sb, in0=ctx_ps[blk][:, 0:P], scalar1=recip)
        dma_engs[blk].dma_start(
            out=out_ap[blk * P : (blk + 1) * P, :], in_=ctx_sb
        )


def build_program(td_n=TD_N):
    nc = bacc.Bacc("TRN2", target_bir_lowering=False, debug=False)
    decT = nc.dram_tensor("decT", [P, td_n], F32, kind="ExternalInput").ap()
    encT = nc.dram_tensor("encT", [P, TE], F16, kind="ExternalInput").ap()
    enc_ones = nc.dram_tensor(
        "enc_ones", [P, N_CHUNK, P + 1], BF16, kind="ExternalInput"
    ).ap()
    coef = nc.dram_tensor(
        "coef", [P, K_MODES, td_n], BF16, kind="ExternalInput"
    ).ap()
    w = nc.dram_tensor("w", [D, D], F32, kind="ExternalInput").ap()
    out = nc.dram_tensor("ctx_out", [td_n, D], F32, kind="ExternalOutput").ap()
    with tile.TileContext(nc) as tc, ExitStack() as ctx:
        _build_body(ctx, tc, out, decT, encT, enc_ones, coef, w)
    nc.compile()
    return nc


def _prep_core_inputs(dec_slice, enc_b, w, v, bf16):
    """Host-side layout marshalling for one core (no reference FLOPs)."""
    decT = np.ascontiguousarray(dec_slice.T)                       # [d, td]
    encT = np.ascontiguousarray(enc_b.T).astype(np.float16)        # [d, te]
    enc_ones = np.ones((P, N_CHUNK, P + 1), dtype=np.float32)
    enc_ones[:, :, :P] = enc_b.reshape(N_CHUNK, P, D).transpose(1, 0, 2)
    # c[d, m] = V_d * b_m broadcast along td
    cmat = (v[:, 0:1] * BCOEF[None, :]).astype(np.float32)         # [d, K]
    coef = np.repeat(cmat[:, :, None], TD_N, axis=2)               # [d, K, td]
    return {
        "decT": decT,
        "encT": encT,
        "enc_ones": enc_ones.astype(bf16),
        "coef": coef.astype(bf16),
        "w": np.ascontiguousarray(w),
    }


_CACHED_NC = None


def _run(inputs, trace=False):
    global _CACHED_NC
    if _CACHED_NC is None:
        _CACHED_NC = build_program()
    nc = _CACHED_NC
    bf16 = mybir.dt.np(BF16)

    dec = np.ascontiguousarray(inputs["decoder_outputs"], dtype=np.float32)
    enc = np.ascontiguousarray(inputs["encoder_outputs"], dtype=np.float32)
    w = np.ascontiguousarray(inputs["W"], dtype=np.float32)
    v = np.ascontiguousarray(inputs["V"], dtype=np.float32)

    in_maps = []
    for c in range(N_CORES):
        b, h = divmod(c, 2)
        in_maps.append(
            _prep_core_inputs(dec[b, h * TD_N : (h + 1) * TD_N], enc[b], w, v, bf16)
        )
    res = run_bass_kernel_spmd(nc, in_maps, core_ids=list(range(N_CORES)), trace=trace)
    out = np.zeros((B, TD, D), dtype=np.float32)
    for c in range(N_CORES):
        b, h = divmod(c, 2)
        out[b, h * TD_N : (h + 1) * TD_N] = res.results[c]["ctx_out"]
    return out, res


def kernel(**inputs):
    out, _ = _run(inputs, trace=False)
    return out


if __name__ == "__main__":
    rng = np.random.default_rng(0)
    inputs = {
        "decoder_outputs": rng.standard_normal((B, TD, D)).astype(np.float32),
        "encoder_outputs": rng.standard_normal((B, TE, D)).astype(np.float32),
        "W": (rng.uniform(-0.15, 0.15, (D, D))).astype(np.float32),
        "V": (rng.uniform(-0.21, 0.21, (D, 1))).astype(np.float32),
    }
    out = kernel(**inputs)
    print("ran, output shape", out.shape)


# revision 22
# speedup vs baseline: 3.3734x; 3.3734x over previous
"""Bahdanau additive-attention kernel for Trainium2, SPMD over 8 NeuronCores.

Reference computation (per batch b):
    dec_t  = dec @ W                                  [TD, D]
    score  = sum_d V[d] * tanh(dec_t[td,d] + enc[te,d])   [TD, TE]
    attn   = softmax(score, axis=te)
    ctx    = attn @ enc                               [TD, D]

Sharding: B=4, 8 cores -> core c handles batch b = c//2, td half h = c%2
(256 td rows each); enc/W replicated per batch. Host side does layout
marshalling only; all FLOPs of the reference computation run on device.

Algorithm: instead of evaluating tanh over the full [TD, TE, D] volume
(16.8M ACT elems/core ~ 110 us), use a Fourier-feature factorization:
    tanh(s) ~ sum_m b_m sin(w_m s),   w_m = (m-1/2)*pi/L,  m=1..K
and since sin(w(x+y)) = sin(wx)cos(wy) + cos(wx)sin(wy), the score
becomes a plain GEMM over an expanded inner dim D*2K:
    score[td,te] = sum_{m,d} V_d b_m [sin_m(a)cos_m(e) + cos_m(a)sin_m(e)]
with a = dec_t, e = enc. Feature work is only (TD_N + TE)*D*2K elems.

HW Sin is valid on [-pi, pi] only and the ISA has no mod op, so phases
are range-reduced via the fp->int16 write conversion, which the HW does
round-to-nearest-even (verified by probe):
    rq  = int16(x * w_m/2pi)                     (tensor_scalar mult)
    d   = x*w_m/2pi - rq      in [-1/2, 1/2]     (scalar_tensor_tensor)
    mask= (d > 1/4) ? 1 : 0                      (tensor_scalar is_gt, batched)
    d2  = (d + 1/4) - mask    in [-1/2, 1/2]     (scalar_tensor_tensor, batched)
then ACT computes Sin(d * 2pi) = sin(w x), Sin(d2 * 2pi) = cos(w x),
both halves in one batched Sin instruction per side (scale=2pi, bias=0).
All elementwise work runs on DVE (~250 G elem/s fp16 tensor_scalar,
~165 G elem/s two-tensor-operand ops, measured); the GpSimd engine is
~30x slower for bulk elementwise and is used only as a DMA queue.
Coefficients c[d,m] = V_d*b_m fold into the (smaller) a-side features
with one two-op tensor_scalar per mode: (fa * V) * b_m.

Score is accumulated TRANSPOSED, scoreT[te, td], via lhsT=G (e-side)
rhs=F (a-side) matmuls, so the softmax epilogue needs no PE transposes:
exp -> escT is already [te, td] = lhsT for the context matmul against
[enc | 1] (denominator in the extra column), then reciprocal * scale.
"""

from contextlib import ExitStack

import numpy as np

import concourse.bacc as bacc
import concourse.tile as tile
from concourse import mybir
from concourse.bass_utils import run_bass_kernel_spmd

F32 = mybir.dt.float32
F16 = mybir.dt.float16
I16 = mybir.dt.int16
BF16 = mybir.dt.bfloat16
ALU = mybir.AluOpType

B, TD, TE, D = 4, 512, 512, 128
N_CORES = 8
TD_N = (B * TD) // N_CORES          # 256 td rows per core
P = 128
N_CHUNK = TE // P                   # 4 te chunks

# --- Fourier fit of tanh on [-L, L], half-integer sine modes ---
K_MODES = 12
FIT_L = 10.5


def _fit_tanh_modes(L=FIT_L, K=K_MODES):
    om = (np.arange(1, K + 1) - 0.5) * np.pi / L
    s = np.linspace(-L, L, 8001)
    w = np.exp(-0.5 * (s / 1.9) ** 2) + 1e-3 * (np.abs(s) < L - 0.7)
    M = np.sin(np.outer(s, om))
    Wt = np.sqrt(w)[:, None]
    b, *_ = np.linalg.lstsq(M * Wt, np.tanh(s) * Wt[:, 0], rcond=None)
    return om.astype(np.float64), b.astype(np.float64)

OMEGA, BCOEF = _fit_tanh_modes()

TWO_PI = float(2 * np.pi)

# modes per pipeline batch (sum == K_MODES); small tail batches shrink the
# PE drain after the last ACT feature instruction
BATCHES = [4, 4, 4]

DEBUG_SCORE = None  # set to a [dram_ap] list in debug builds


def _build_body(ctx, tc, out_ap, decT_ap, encT_ap, enc_ones_ap, v_ap, w_ap):
    nc = tc.nc

    consts = ctx.enter_context(tc.tile_pool(name="consts", bufs=1))
    setup_ps = ctx.enter_context(tc.tile_pool(name="setup_ps", bufs=1, space="PSUM"))
    st_ps_pool = ctx.enter_context(tc.tile_pool(name="st_ps", bufs=1, space="PSUM"))
    ctx_ps_pool = ctx.enter_context(tc.tile_pool(name="ctx_ps", bufs=1, space="PSUM"))
    de_pool = ctx.enter_context(tc.tile_pool(name="de", bufs=2))
    rqe_pool = ctx.enter_context(tc.tile_pool(name="rqe", bufs=2))
    me_pool = ctx.enter_context(tc.tile_pool(name="me", bufs=2))
    da_pool = ctx.enter_context(tc.tile_pool(name="da", bufs=2))
    rqa_pool = ctx.enter_context(tc.tile_pool(name="rqa", bufs=2))
    ma_pool = ctx.enter_context(tc.tile_pool(name="ma", bufs=2))
    ge_pool = ctx.enter_context(tc.tile_pool(name="ge", bufs=2))
    fa_pool = ctx.enter_context(tc.tile_pool(name="fa", bufs=2))
    fs_pool = ctx.enter_context(tc.tile_pool(name="fs", bufs=2))
    esc_pool = ctx.enter_context(tc.tile_pool(name="esc", bufs=4))
    out_pool = ctx.enter_context(tc.tile_pool(name="outp", bufs=2))

    # ---- input DMAs ----
    encT = consts.tile([P, TE], F16)              # [d, te] fp16 phases src
    nc.sync.dma_start(out=encT, in_=encT_ap)
    decT = consts.tile([P, TD_N], F32)            # [d, td] fp32
    nc.scalar.dma_start(out=decT, in_=decT_ap)
    w_sb = consts.tile([P, P], F32)
    nc.scalar.dma_start(out=w_sb, in_=w_ap)
    v_sb = consts.tile([P, 1], F32)               # V (fp32, per-partition)
    nc.gpsimd.dma_start(out=v_sb, in_=v_ap)
    enc_ones = consts.tile([P, N_CHUNK, P + 1], BF16)   # [te | 1.0]
    nc.gpsimd.dma_start(out=enc_ones, in_=enc_ones_ap)

    # ---- dec_t = (dec @ W)^T on PE (fp32 for phase accuracy) ----
    dec_t_ps = setup_ps.tile([P, TD_N], F32)
    nc.tensor.matmul(dec_t_ps, w_sb, decT, start=True, stop=True)
    dec_tT = consts.tile([P, TD_N], F32)
    nc.vector.tensor_copy(dec_tT, dec_t_ps)

    # scoreT accumulator: [te(4x128), td] fp32. Each te chunk gets its own
    # 2KB PSUM bank (start_tensor_calc zeroes the whole bank, so concurrent
    # accumulation groups must not share one); cols TD_N..511 are padding.
    st_full = st_ps_pool.tile([P, N_CHUNK, 512], F32)
    st = st_full[:, :, 0:TD_N]

    n_mm = 2 * K_MODES  # accumulating matmuls per chunk
    mm_i = 0
    m0 = 0
    for bs in BATCHES:
        # phase args, mode-major layout [d, mode, half, n]:
        #   [:, j, 0, :] = d  = q - round(q),          q = x*w_m/2pi
        #   [:, j, 1, :] = d2 = (d + 1/4) - (d > 1/4)  (cos arg)
        de = de_pool.tile([P, bs, 2, TE], F16, tag="de")
        rqe = rqe_pool.tile([P, bs, TE], I16, tag="rqe")
        for j in range(bs):
            sc = float(OMEGA[m0 + j] / (2 * np.pi))
            nc.vector.tensor_scalar(
                out=rqe[:, j], in0=encT, scalar1=sc, scalar2=None, op0=ALU.mult
            )
            nc.vector.scalar_tensor_tensor(
                out=de[:, j, 0], in0=encT, scalar=sc, in1=rqe[:, j],
                op0=ALU.mult, op1=ALU.subtract,
            )
        maske = me_pool.tile([P, bs, TE], F16, tag="maske")
        nc.vector.tensor_scalar(
            out=maske, in0=de[:, :, 0], scalar1=0.25, scalar2=None, op0=ALU.is_gt
        )
        nc.vector.scalar_tensor_tensor(
            out=de[:, :, 1], in0=de[:, :, 0], scalar=0.25, in1=maske,
            op0=ALU.add, op1=ALU.subtract,
        )

        da = da_pool.tile([P, bs, 2, TD_N], F16, tag="da")
        rqa = rqa_pool.tile([P, bs, TD_N], I16, tag="rqa")
        for j in range(bs):
            sc = float(OMEGA[m0 + j] / (2 * np.pi))
            nc.vector.tensor_scalar(
                out=rqa[:, j], in0=dec_tT, scalar1=sc, scalar2=None, op0=ALU.mult
            )
            nc.vector.scalar_tensor_tensor(
                out=da[:, j, 0], in0=dec_tT, scalar=sc, in1=rqa[:, j],
                op0=ALU.mult, op1=ALU.subtract,
            )
        maska = ma_pool.tile([P, bs, TD_N], F16, tag="maska")
        nc.vector.tensor_scalar(
            out=maska, in0=da[:, :, 0], scalar1=0.25, scalar2=None, op0=ALU.is_gt
        )
        nc.vector.scalar_tensor_tensor(
            out=da[:, :, 1], in0=da[:, :, 0], scalar=0.25, in1=maska,
            op0=ALU.add, op1=ALU.subtract,
        )

        # features: ge[:, j, 0, :] = sin(w_m e), ge[:, j, 1, :] = cos(w_m e)
        ge = ge_pool.tile([P, bs, 2, TE], BF16, tag="ge")
        nc.scalar.activation(
            out=ge, in_=de, func=mybir.ActivationFunctionType.Sin, scale=TWO_PI
        )
        fa = fa_pool.tile([P, bs, 2, TD_N], BF16, tag="fa")
        nc.scalar.activation(
            out=fa, in_=da, func=mybir.ActivationFunctionType.Sin, scale=TWO_PI
        )

        # fold c[d,m] = V_d*b_m into both a-side halves: (fa * V) * b_m
        fsc = fs_pool.tile([P, bs, 2, TD_N], BF16, tag="fsc")
        for j in range(bs):
            nc.vector.tensor_scalar(
                out=fsc[:, j], in0=fa[:, j], scalar1=v_sb,
                scalar2=float(BCOEF[m0 + j]), op0=ALU.mult, op1=ALU.mult,
            )

        # scoreT += G_sin^T F~_cos + G_cos^T F~_sin
        for j in range(bs):
            for half in (0, 1):
                for c in range(N_CHUNK):
                    nc.tensor.matmul(
                        st[:, c, :],
                        ge[:, j, half, c * P : (c + 1) * P],
                        fsc[:, j, 1 - half, :],
                        start=(mm_i == 0),
                        stop=(mm_i == n_mm - 1),
                        skip_group_check=True,
                    )
                mm_i += 1
        m0 += bs

    if DEBUG_SCORE is not None:
        dbg = consts.tile([P, N_CHUNK, TD_N], F32)
        nc.vector.tensor_copy(dbg, st)
        nc.sync.dma_start(out=DEBUG_SCORE[0], in_=dbg)

    # ---- softmax + context, chunk-staggered ----
    # one full PSUM bank per block: the two blocks' accumulation groups
    # interleave, so they must not share a bank
    ctx_ps = [
        ctx_ps_pool.tile([P, 512], F32, tag=f"ctx{b}", name=f"ctx_ps{b}")[:, 0 : P + 1]
        for b in range(2)
    ]
    for c in range(N_CHUNK):
        escT = esc_pool.tile([P, TD_N], BF16, tag=f"escT{c}")
        nc.scalar.activation(
            out=escT, in_=st[:, c, :], func=mybir.ActivationFunctionType.Exp
        )
        for blk in range(2):
            nc.tensor.matmul(
                ctx_ps[blk], escT[:, blk * P : (blk + 1) * P], enc_ones[:, c, :],
                start=(c == 0), stop=(c == N_CHUNK - 1),
            )
    dma_engs = [nc.sync, nc.scalar]
    for blk in range(2):
        recip = out_pool.tile([P, 1], F32, tag=f"recip{blk}")
        nc.vector.reciprocal(recip, ctx_ps[blk][:, P : P + 1])
        ctx_sb = out_pool.tile([P, P], F32, tag=f"ctx_sb{blk}")
        nc.vector.tensor_scalar_mul(out=ctx_sb, in0=ctx_ps[blk][:, 0:P], scalar1=recip)
        dma_engs[blk].dma_start(
            out=out_ap[blk * P : (blk + 1) * P, :], in_=ctx_sb
        )


def build_program(td_n=TD_N):
    nc = bacc.Bacc("TRN2", target_bir_lowering=False, debug=False)
    decT = nc.dram_tensor("decT", [P, td_n], F32, kind="ExternalInput").ap()
    encT = nc.dram_tensor("encT", [P, TE], F16, kind="ExternalInput").ap()
    enc_ones = nc.dram_tensor(
        "enc_ones", [P, N_CHUNK, P + 1], BF16, kind="ExternalInput"
    ).ap()
    v = nc.dram_tensor("v", [P, 1], F32, kind="ExternalInput").ap()
    w = nc.dram_tensor("w", [D, D], F32, kind="ExternalInput").ap()
    out = nc.dram_tensor("ctx_out", [td_n, D], F32, kind="ExternalOutput").ap()
    with tile.TileContext(nc) as tc, ExitStack() as ctx:
        _build_body(ctx, tc, out, decT, encT, enc_ones, v, w)
    nc.compile()
    return nc


def _prep_core_inputs(dec_slice, enc_b, w, v, bf16):
    """Host-side layout marshalling for one core (no reference FLOPs)."""
    decT = np.ascontiguousarray(dec_slice.T)                       # [d, td]
    encT = np.ascontiguousarray(enc_b.T).astype(np.float16)        # [d, te]
    enc_ones = np.ones((P, N_CHUNK, P + 1), dtype=np.float32)
    enc_ones[:, :, :P] = enc_b.reshape(N_CHUNK, P, D).transpose(1, 0, 2)
    return {
        "decT": decT,
        "encT": encT,
        "enc_ones": enc_ones.astype(bf16),
        "v": np.ascontiguousarray(v, dtype=np.float32),
        "w": np.ascontiguousarray(w),
    }


_CACHED_NC = None


def _run(inputs, trace=False):
    global _CACHED_NC
    if _CACHED_NC is None:
        _CACHED_NC = build_program()
    nc = _CACHED_NC
    bf16 = mybir.dt.np(BF16)

    dec = np.ascontiguousarray(inputs["decoder_outputs"], dtype=np.float32)
    enc = np.ascontiguousarray(inputs["encoder_outputs"], dtype=np.float32)
    w = np.ascontiguousarray(inputs["W"], dtype=np.float32)
    v = np.ascontiguousarray(inputs["V"], dtype=np.float32)

    in_maps = []
    for c in range(N_CORES):
        b, h = divmod(c, 2)
        in_maps.append(
            _prep_core_inputs(dec[b, h * TD_N : (h + 1) * TD_N], enc[b], w, v, bf16)
        )
    res = run_bass_kernel_spmd(nc, in_maps, core_ids=list(range(N_CORES)), trace=trace)
    out = np.zeros((B, TD, D), dtype=np.float32)
    for c in range(N_CORES):
        b, h = divmod(c, 2)
        out[b, h * TD_N : (h + 1) * TD_N] = res.results[c]["ctx_out"]
    return out, res


def kernel(**inputs):
    out, _ = _run(inputs, trace=False)
    return out


if __name__ == "__main__":
    rng = np.random.default_rng(0)
    inputs = {
        "decoder_outputs": rng.standard_normal((B, TD, D)).astype(np.float32),
        "encoder_outputs": rng.standard_normal((B, TE, D)).astype(np.float32),
        "W": (rng.uniform(-0.15, 0.15, (D, D))).astype(np.float32),
        "V": (rng.uniform(-0.21, 0.21, (D, 1))).astype(np.float32),
    }
    out = kernel(**inputs)
    print("ran, output shape", out.shape)


# revision 23
# speedup vs baseline: 3.9650x; 1.1754x over previous
"""Bahdanau additive-attention kernel for Trainium2, SPMD over 8 NeuronCores.

Reference computation (per batch b):
    dec_t  = dec @ W                                  [TD, D]
    score  = sum_d V[d] * tanh(dec_t[td,d] + enc[te,d])   [TD, TE]
    attn   = softmax(score, axis=te)
    ctx    = attn @ enc                               [TD, D]

Sharding: B=4, 8 cores -> core c handles batch b = c//2, td half h = c%2
(256 td rows each); enc/W replicated per batch. Host side does layout
marshalling only; all FLOPs of the reference computation run on device.

Algorithm: instead of evaluating tanh over the full [TD, TE, D] volume
(16.8M ACT elems/core ~ 110 us), use a Fourier-feature factorization:
    tanh(s) ~ sum_m b_m sin(w_m s),   w_m = (m-1/2)*pi/L,  m=1..K
and since sin(w(x+y)) = sin(wx)cos(wy) + cos(wx)sin(wy), the score
becomes a plain GEMM over an expanded inner dim D*2K:
    score[td,te] = sum_{m,d} V_d b_m [sin_m(a)cos_m(e) + cos_m(a)sin_m(e)]
with a = dec_t, e = enc. Feature work is only (TD_N + TE)*D*2K elems.

HW Sin is valid on [-pi, pi] only and the ISA has no mod op, so phases
are range-reduced via the fp->int16 write conversion, which the HW does
round-to-nearest-even (verified by probe):
    rq  = int16(x * w_m/2pi)                     (tensor_scalar mult)
    d   = x*w_m/2pi - rq      in [-1/2, 1/2]     (scalar_tensor_tensor)
    mask= (d > 1/4) ? 1 : 0                      (tensor_scalar is_gt, batched)
    d2  = (d + 1/4) - mask    in [-1/2, 1/2]     (scalar_tensor_tensor, batched)
then ACT computes Sin(d * 2pi) = sin(w x), Sin(d2 * 2pi) = cos(w x),
both halves in one batched Sin instruction per side (scale=2pi, bias=0).
All elementwise work runs on DVE (~250 G elem/s fp16 tensor_scalar,
~165 G elem/s two-tensor-operand ops, measured); the GpSimd engine is
~30x slower for bulk elementwise and is used only as a DMA queue.
Coefficients c[d,m] = V_d*b_m fold into the (smaller) a-side features
with one two-op tensor_scalar per mode: (fa * V) * b_m.

Score is accumulated TRANSPOSED, scoreT[te, td], via lhsT=G (e-side)
rhs=F (a-side) matmuls, so the softmax epilogue needs no PE transposes:
exp -> escT is already [te, td] = lhsT for the context matmul against
[enc | 1] (denominator in the extra column), then reciprocal * scale.
"""

from contextlib import ExitStack

import numpy as np

import concourse.bacc as bacc
import concourse.tile as tile
from concourse import mybir
from concourse.bass_utils import run_bass_kernel_spmd

F32 = mybir.dt.float32
F16 = mybir.dt.float16
I16 = mybir.dt.int16
BF16 = mybir.dt.bfloat16
ALU = mybir.AluOpType

B, TD, TE, D = 4, 512, 512, 128
N_CORES = 8
TD_N = (B * TD) // N_CORES          # 256 td rows per core
P = 128
N_CHUNK = TE // P                   # 4 te chunks

# --- Fourier fit of tanh on [-L, L], half-integer sine modes ---
K_MODES = 9
FIT_L = 8.6


def _fit_tanh_modes(L=FIT_L, K=K_MODES):
    om = (np.arange(1, K + 1) - 0.5) * np.pi / L
    s = np.linspace(-L, L, 8001)
    w = np.exp(-0.5 * (s / 1.9) ** 2) + 1e-3 * (np.abs(s) < L - 0.7)
    M = np.sin(np.outer(s, om))
    Wt = np.sqrt(w)[:, None]
    b, *_ = np.linalg.lstsq(M * Wt, np.tanh(s) * Wt[:, 0], rcond=None)
    return om.astype(np.float64), b.astype(np.float64)

OMEGA, BCOEF = _fit_tanh_modes()

TWO_PI = float(2 * np.pi)

# modes per pipeline batch (sum == K_MODES); small tail batches shrink the
# PE drain after the last ACT feature instruction
BATCHES = [4, 4, 1]

DEBUG_SCORE = None  # set to a [dram_ap] list in debug builds


def _build_body(ctx, tc, out_ap, decT_ap, encT_ap, enc_ones_ap, v_ap, w_ap):
    nc = tc.nc

    consts = ctx.enter_context(tc.tile_pool(name="consts", bufs=1))
    setup_ps = ctx.enter_context(tc.tile_pool(name="setup_ps", bufs=1, space="PSUM"))
    st_ps_pool = ctx.enter_context(tc.tile_pool(name="st_ps", bufs=1, space="PSUM"))
    ctx_ps_pool = ctx.enter_context(tc.tile_pool(name="ctx_ps", bufs=1, space="PSUM"))
    de_pool = ctx.enter_context(tc.tile_pool(name="de", bufs=2))
    rqe_pool = ctx.enter_context(tc.tile_pool(name="rqe", bufs=2))
    me_pool = ctx.enter_context(tc.tile_pool(name="me", bufs=2))
    da_pool = ctx.enter_context(tc.tile_pool(name="da", bufs=2))
    rqa_pool = ctx.enter_context(tc.tile_pool(name="rqa", bufs=2))
    ma_pool = ctx.enter_context(tc.tile_pool(name="ma", bufs=2))
    ge_pool = ctx.enter_context(tc.tile_pool(name="ge", bufs=2))
    fa_pool = ctx.enter_context(tc.tile_pool(name="fa", bufs=2))
    fs_pool = ctx.enter_context(tc.tile_pool(name="fs", bufs=2))
    esc_pool = ctx.enter_context(tc.tile_pool(name="esc", bufs=4))
    out_pool = ctx.enter_context(tc.tile_pool(name="outp", bufs=2))

    # ---- input DMAs ----
    encT = consts.tile([P, TE], F16)              # [d, te] fp16 phases src
    nc.sync.dma_start(out=encT, in_=encT_ap)
    decT = consts.tile([P, TD_N], F32)            # [d, td] fp32
    nc.scalar.dma_start(out=decT, in_=decT_ap)
    w_sb = consts.tile([P, P], F32)
    nc.scalar.dma_start(out=w_sb, in_=w_ap)
    v_sb = consts.tile([P, 1], F32)               # V (fp32, per-partition)
    nc.gpsimd.dma_start(out=v_sb, in_=v_ap)
    enc_ones = consts.tile([P, N_CHUNK, P + 1], BF16)   # [te | 1.0]
    nc.gpsimd.dma_start(out=enc_ones, in_=enc_ones_ap)

    # ---- dec_t = (dec @ W)^T on PE (fp32 for phase accuracy) ----
    dec_t_ps = setup_ps.tile([P, TD_N], F32)
    nc.tensor.matmul(dec_t_ps, w_sb, decT, start=True, stop=True)
    dec_tT = consts.tile([P, TD_N], F32)
    nc.vector.tensor_copy(dec_tT, dec_t_ps)

    # scoreT accumulator: [te(4x128), td] fp32. Each te chunk gets its own
    # 2KB PSUM bank (start_tensor_calc zeroes the whole bank, so concurrent
    # accumulation groups must not share one); cols TD_N..511 are padding.
    st_full = st_ps_pool.tile([P, N_CHUNK, 512], F32)
    st = st_full[:, :, 0:TD_N]

    n_mm = 2 * K_MODES  # accumulating matmuls per chunk
    mm_i = 0
    m0 = 0
    for bs in BATCHES:
        # phase args, mode-major layout [d, mode, half, n]:
        #   [:, j, 0, :] = d  = q - round(q),          q = x*w_m/2pi
        #   [:, j, 1, :] = d2 = (d + 1/4) - (d > 1/4)  (cos arg)
        de = de_pool.tile([P, bs, 2, TE], F16, tag="de")
        rqe = rqe_pool.tile([P, bs, TE], I16, tag="rqe")
        for j in range(bs):
            sc = float(OMEGA[m0 + j] / (2 * np.pi))
            nc.vector.tensor_scalar(
                out=rqe[:, j], in0=encT, scalar1=sc, scalar2=None, op0=ALU.mult
            )
            nc.vector.scalar_tensor_tensor(
                out=de[:, j, 0], in0=encT, scalar=sc, in1=rqe[:, j],
                op0=ALU.mult, op1=ALU.subtract,
            )
        maske = me_pool.tile([P, bs, TE], F16, tag="maske")
        nc.vector.tensor_scalar(
            out=maske, in0=de[:, :, 0], scalar1=0.25, scalar2=None, op0=ALU.is_gt
        )
        nc.vector.scalar_tensor_tensor(
            out=de[:, :, 1], in0=de[:, :, 0], scalar=0.25, in1=maske,
            op0=ALU.add, op1=ALU.subtract,
        )

        da = da_pool.tile([P, bs, 2, TD_N], F16, tag="da")
        rqa = rqa_pool.tile([P, bs, TD_N], I16, tag="rqa")
        for j in range(bs):
            sc = float(OMEGA[m0 + j] / (2 * np.pi))
            nc.vector.tensor_scalar(
                out=rqa[:, j], in0=dec_tT, scalar1=sc, scalar2=None, op0=ALU.mult
            )
            nc.vector.scalar_tensor_tensor(
                out=da[:, j, 0], in0=dec_tT, scalar=sc, in1=rqa[:, j],
                op0=ALU.mult, op1=ALU.subtract,
            )
        maska = ma_pool.tile([P, bs, TD_N], F16, tag="maska")
        nc.vector.tensor_scalar(
            out=maska, in0=da[:, :, 0], scalar1=0.25, scalar2=None, op0=ALU.is_gt
        )
        nc.vector.scalar_tensor_tensor(
            out=da[:, :, 1], in0=da[:, :, 0], scalar=0.25, in1=maska,
            op0=ALU.add, op1=ALU.subtract,
        )

        # features: ge[:, j, 0, :] = sin(w_m e), ge[:, j, 1, :] = cos(w_m e)
        ge = ge_pool.tile([P, bs, 2, TE], BF16, tag="ge")
        nc.scalar.activation(
            out=ge, in_=de, func=mybir.ActivationFunctionType.Sin, scale=TWO_PI
        )
        fa = fa_pool.tile([P, bs, 2, TD_N], BF16, tag="fa")
        nc.scalar.activation(
            out=fa, in_=da, func=mybir.ActivationFunctionType.Sin, scale=TWO_PI
        )

        # fold c[d,m] = V_d*b_m into both a-side halves: (fa * V) * b_m
        fsc = fs_pool.tile([P, bs, 2, TD_N], BF16, tag="fsc")
        for j in range(bs):
            nc.vector.tensor_scalar(
                out=fsc[:, j], in0=fa[:, j], scalar1=v_sb,
                scalar2=float(BCOEF[m0 + j]), op0=ALU.mult, op1=ALU.mult,
            )

        # scoreT += G_sin^T F~_cos + G_cos^T F~_sin
        for j in range(bs):
            for half in (0, 1):
                for c in range(N_CHUNK):
                    nc.tensor.matmul(
                        st[:, c, :],
                        ge[:, j, half, c * P : (c + 1) * P],
                        fsc[:, j, 1 - half, :],
                        start=(mm_i == 0),
                        stop=(mm_i == n_mm - 1),
                        skip_group_check=True,
                    )
                mm_i += 1
        m0 += bs

    if DEBUG_SCORE is not None:
        dbg = consts.tile([P, N_CHUNK, TD_N], F32)
        nc.vector.tensor_copy(dbg, st)
        nc.sync.dma_start(out=DEBUG_SCORE[0], in_=dbg)

    # ---- softmax + context, chunk-staggered ----
    # one full PSUM bank per block: the two blocks' accumulation groups
    # interleave, so they must not share a bank
    ctx_ps = [
        ctx_ps_pool.tile([P, 512], F32, tag=f"ctx{b}", name=f"ctx_ps{b}")[:, 0 : P + 1]
        for b in range(2)
    ]
    for c in range(N_CHUNK):
        escT = esc_pool.tile([P, TD_N], BF16, tag=f"escT{c}")
        nc.scalar.activation(
            out=escT, in_=st[:, c, :], func=mybir.ActivationFunctionType.Exp
        )
        for blk in range(2):
            nc.tensor.matmul(
                ctx_ps[blk], escT[:, blk * P : (blk + 1) * P], enc_ones[:, c, :],
                start=(c == 0), stop=(c == N_CHUNK - 1),
            )
    dma_engs = [nc.sync, nc.scalar]
    for blk in range(2):
        recip = out_pool.tile([P, 1], F32, tag=f"recip{blk}")
        nc.vector.reciprocal(recip, ctx_ps[blk][:, P : P + 1])
        ctx_sb = out_pool.tile([P, P], F32, tag=f"ctx_sb{blk}")
        nc.vector.tensor_scalar_mul(out=ctx_sb, in0=ctx_ps[blk][:, 0:P], scalar1=recip)
        dma_engs[blk].dma_start(
            out=out_ap[blk * P : (blk + 1) * P, :], in_=ctx_sb
        )


def build_program(td_n=TD_N):
    nc = bacc.Bacc("TRN2", target_bir_lowering=False, debug=False)
    decT = nc.dram_tensor("decT", [P, td_n], F32, kind="ExternalInput").ap()
    encT = nc.dram_tensor("encT", [P, TE], F16, kind="ExternalInput").ap()
    enc_ones = nc.dram_tensor(
        "enc_ones", [P, N_CHUNK, P + 1], BF16, kind="ExternalInput"
    ).ap()
    v = nc.dram_tensor("v", [P, 1], F32, kind="ExternalInput").ap()
    w = nc.dram_tensor("w", [D, D], F32, kind="ExternalInput").ap()
    out = nc.dram_tensor("ctx_out", [td_n, D], F32, kind="ExternalOutput").ap()
    with tile.TileContext(nc) as tc, ExitStack() as ctx:
        _build_body(ctx, tc, out, decT, encT, enc_ones, v, w)
    nc.compile()
    return nc


def _prep_core_inputs(dec_slice, enc_b, w, v, bf16):
    """Host-side layout marshalling for one core (no reference FLOPs)."""
    decT = np.ascontiguousarray(dec_slice.T)                       # [d, td]
    encT = np.ascontiguousarray(enc_b.T).astype(np.float16)        # [d, te]
    enc_ones = np.ones((P, N_CHUNK, P + 1), dtype=np.float32)
    enc_ones[:, :, :P] = enc_b.reshape(N_CHUNK, P, D).transpose(1, 0, 2)
    return {
        "decT": decT,
        "encT": encT,
        "enc_ones": enc_ones.astype(bf16),
        "v": np.ascontiguousarray(v, dtype=np.float32),
        "w": np.ascontiguousarray(w),
    }


_CACHED_NC = None


def _run(inputs, trace=False):
    global _CACHED_NC
    if _CACHED_NC is None:
        _CACHED_NC = build_program()
    nc = _CACHED_NC
    bf16 = mybir.dt.np(BF16)

    dec = np.ascontiguousarray(inputs["decoder_outputs"], dtype=np.float32)
    enc = np.ascontiguousarray(inputs["encoder_outputs"], dtype=np.float32)
    w = np.ascontiguousarray(inputs["W"], dtype=np.float32)
    v = np.ascontiguousarray(inputs["V"], dtype=np.float32)

    in_maps = []
    for c in range(N_CORES):
        b, h = divmod(c, 2)
        in_maps.append(
            _prep_core_inputs(dec[b, h * TD_N : (h + 1) * TD_N], enc[b], w, v, bf16)
        )
    res = run_bass_kernel_spmd(nc, in_maps, core_ids=list(range(N_CORES)), trace=trace)
    out = np.zeros((B, TD, D), dtype=np.float32)
    for c in range(N_CORES):
        b, h = divmod(c, 2)
        out[b, h * TD_N : (h + 1) * TD_N] = res.results[c]["ctx_out"]
    return out, res


def kernel(**inputs):
    out, _ = _run(inputs, trace=False)
    return out


if __name__ == "__main__":
    rng = np.random.default_rng(0)
    inputs = {
        "decoder_outputs": rng.standard_normal((B, TD, D)).astype(np.float32),
        "encoder_outputs": rng.standard_normal((B, TE, D)).astype(np.float32),
        "W": (rng.uniform(-0.15, 0.15, (D, D))).astype(np.float32),
        "V": (rng.uniform(-0.21, 0.21, (D, 1))).astype(np.float32),
    }
    out = kernel(**inputs)
    print("ran, output shape", out.shape)


# revision 24
# speedup vs baseline: 4.2427x; 1.0700x over previous
"""Bahdanau additive-attention kernel for Trainium2, SPMD over 8 NeuronCores.

Reference computation (per batch b):
    dec_t  = dec @ W                                  [TD, D]
    score  = sum_d V[d] * tanh(dec_t[td,d] + enc[te,d])   [TD, TE]
    attn   = softmax(score, axis=te)
    ctx    = attn @ enc                               [TD, D]

Sharding: B=4, 8 cores -> core c handles batch b = c//2, td half h = c%2
(256 td rows each); enc/W replicated per batch. Host side does layout
marshalling only; all FLOPs of the reference computation run on device.

Algorithm: instead of evaluating tanh over the full [TD, TE, D] volume
(16.8M ACT elems/core ~ 110 us), use a Fourier-feature factorization:
    tanh(s) ~ sum_m b_m sin(w_m s),   w_m = (m-1/2)*pi/L,  m=1..K
and since sin(w(x+y)) = sin(wx)cos(wy) + cos(wx)sin(wy), the score
becomes a plain GEMM over an expanded inner dim D*2K:
    score[td,te] = sum_{m,d} V_d b_m [sin_m(a)cos_m(e) + cos_m(a)sin_m(e)]
with a = dec_t, e = enc. Feature work is only (TD_N + TE)*D*2K elems.

HW Sin is valid on [-pi, pi] only and the ISA has no mod op, so phases
are range-reduced via the fp->int16 write conversion, which the HW does
round-to-nearest-even (verified by probe):
    rq  = int16(x * w_m/2pi)                     (tensor_scalar mult)
    d   = x*w_m/2pi - rq      in [-1/2, 1/2]     (scalar_tensor_tensor)
    mask= (d > 1/4) ? 1 : 0                      (tensor_scalar is_gt, batched)
    d2  = (d + 1/4) - mask    in [-1/2, 1/2]     (scalar_tensor_tensor, batched)
then ACT computes Sin(d * 2pi) = sin(w x), Sin(d2 * 2pi) = cos(w x),
both halves in one batched Sin instruction per side (scale=2pi, bias=0).
All elementwise work runs on DVE (~250 G elem/s fp16 tensor_scalar,
~165 G elem/s two-tensor-operand ops, measured); the GpSimd engine is
~30x slower for bulk elementwise and is used only as a DMA queue.
Coefficients c[d,m] = V_d*b_m fold into the (smaller) a-side features
with one two-op tensor_scalar per mode: (fa * V) * b_m.

Score is accumulated TRANSPOSED, scoreT[te, td], via lhsT=G (e-side)
rhs=F (a-side) matmuls, so the softmax epilogue needs no PE transposes:
exp -> escT is already [te, td] = lhsT for the context matmul against
[enc | 1] (denominator in the extra column), then reciprocal * scale.
"""

from contextlib import ExitStack

import numpy as np

import concourse.bacc as bacc
import concourse.tile as tile
from concourse import mybir
from concourse.bass_utils import run_bass_kernel_spmd

F32 = mybir.dt.float32
F16 = mybir.dt.float16
I16 = mybir.dt.int16
BF16 = mybir.dt.bfloat16
ALU = mybir.AluOpType

B, TD, TE, D = 4, 512, 512, 128
N_CORES = 8
TD_N = (B * TD) // N_CORES          # 256 td rows per core
P = 128
N_CHUNK = TE // P                   # 4 te chunks

# --- Fourier fit of tanh on [-L, L], half-integer sine modes ---
K_MODES = 9
FIT_L = 8.6


def _fit_tanh_modes(L=FIT_L, K=K_MODES):
    om = (np.arange(1, K + 1) - 0.5) * np.pi / L
    s = np.linspace(-L, L, 8001)
    w = np.exp(-0.5 * (s / 1.9) ** 2) + 1e-3 * (np.abs(s) < L - 0.7)
    M = np.sin(np.outer(s, om))
    Wt = np.sqrt(w)[:, None]
    b, *_ = np.linalg.lstsq(M * Wt, np.tanh(s) * Wt[:, 0], rcond=None)
    return om.astype(np.float64), b.astype(np.float64)

OMEGA, BCOEF = _fit_tanh_modes()

TWO_PI = float(2 * np.pi)

# modes per pipeline batch (sum == K_MODES); small tail batches shrink the
# PE drain after the last ACT feature instruction
BATCHES = [3, 3, 2, 1]

DEBUG_SCORE = None  # set to a [dram_ap] list in debug builds


def _build_body(ctx, tc, out_ap, decT_ap, encT_ap, enc_ones_ap, v_ap, w_ap):
    nc = tc.nc

    consts = ctx.enter_context(tc.tile_pool(name="consts", bufs=1))
    setup_ps = ctx.enter_context(tc.tile_pool(name="setup_ps", bufs=1, space="PSUM"))
    st_ps_pool = ctx.enter_context(tc.tile_pool(name="st_ps", bufs=1, space="PSUM"))
    ctx_ps_pool = ctx.enter_context(tc.tile_pool(name="ctx_ps", bufs=1, space="PSUM"))
    de_pool = ctx.enter_context(tc.tile_pool(name="de", bufs=2))
    rqe_pool = ctx.enter_context(tc.tile_pool(name="rqe", bufs=2))
    me_pool = ctx.enter_context(tc.tile_pool(name="me", bufs=2))
    da_pool = ctx.enter_context(tc.tile_pool(name="da", bufs=2))
    rqa_pool = ctx.enter_context(tc.tile_pool(name="rqa", bufs=2))
    ma_pool = ctx.enter_context(tc.tile_pool(name="ma", bufs=2))
    ge_pool = ctx.enter_context(tc.tile_pool(name="ge", bufs=2))
    fa_pool = ctx.enter_context(tc.tile_pool(name="fa", bufs=2))
    fs_pool = ctx.enter_context(tc.tile_pool(name="fs", bufs=2))
    esc_pool = ctx.enter_context(tc.tile_pool(name="esc", bufs=4))
    out_pool = ctx.enter_context(tc.tile_pool(name="outp", bufs=2))

    # ---- input DMAs (dec path first: the dec_t matmul gates a-side work) ----
    decT = consts.tile([P, TD_N], F32)            # [d, td] fp32
    nc.sync.dma_start(out=decT, in_=decT_ap)
    w_sb = consts.tile([P, P], F32)
    nc.sync.dma_start(out=w_sb, in_=w_ap)
    encT = consts.tile([P, TE], F16)              # [d, te] fp16 phases src
    nc.scalar.dma_start(out=encT, in_=encT_ap)
    v_sb = consts.tile([P, 1], F32)               # V (fp32, per-partition)
    nc.gpsimd.dma_start(out=v_sb, in_=v_ap)
    enc_ones = consts.tile([P, N_CHUNK, P + 1], BF16)   # [te | 1.0]
    nc.gpsimd.dma_start(out=enc_ones, in_=enc_ones_ap)

    # ---- dec_t = (dec @ W)^T on PE (fp32 for phase accuracy) ----
    dec_t_ps = setup_ps.tile([P, TD_N], F32)
    nc.tensor.matmul(dec_t_ps, w_sb, decT, start=True, stop=True)
    dec_tT = consts.tile([P, TD_N], F16)
    nc.vector.tensor_copy(dec_tT, dec_t_ps)

    # scoreT accumulator: [te(4x128), td] fp32. Each te chunk gets its own
    # 2KB PSUM bank (start_tensor_calc zeroes the whole bank, so concurrent
    # accumulation groups must not share one); cols TD_N..511 are padding.
    st_full = st_ps_pool.tile([P, N_CHUNK, 512], F32)
    st = st_full[:, :, 0:TD_N]

    n_mm = 2 * K_MODES  # accumulating matmuls per chunk
    mm_i = 0
    m0 = 0
    for bs in BATCHES:
        # phase args, mode-major layout [d, mode, half, n]:
        #   [:, j, 0, :] = d  = q - round(q),          q = x*w_m/2pi
        #   [:, j, 1, :] = d2 = (d + 1/4) - (d > 1/4)  (cos arg)
        da = da_pool.tile([P, bs, 2, TD_N], F16, tag="da")
        rqa = rqa_pool.tile([P, bs, TD_N], I16, tag="rqa")
        for j in range(bs):
            sc = float(OMEGA[m0 + j] / (2 * np.pi))
            nc.vector.tensor_scalar(
                out=rqa[:, j], in0=dec_tT, scalar1=sc, scalar2=None, op0=ALU.mult
            )
            nc.vector.scalar_tensor_tensor(
                out=da[:, j, 0], in0=dec_tT, scalar=sc, in1=rqa[:, j],
                op0=ALU.mult, op1=ALU.subtract,
            )
        maska = ma_pool.tile([P, bs, TD_N], F16, tag="maska")
        nc.vector.tensor_scalar(
            out=maska, in0=da[:, :, 0], scalar1=0.25, scalar2=None, op0=ALU.is_gt
        )
        nc.vector.scalar_tensor_tensor(
            out=da[:, :, 1], in0=da[:, :, 0], scalar=0.25, in1=maska,
            op0=ALU.add, op1=ALU.subtract,
        )

        de = de_pool.tile([P, bs, 2, TE], F16, tag="de")
        rqe = rqe_pool.tile([P, bs, TE], I16, tag="rqe")
        for j in range(bs):
            sc = float(OMEGA[m0 + j] / (2 * np.pi))
            nc.vector.tensor_scalar(
                out=rqe[:, j], in0=encT, scalar1=sc, scalar2=None, op0=ALU.mult
            )
            nc.vector.scalar_tensor_tensor(
                out=de[:, j, 0], in0=encT, scalar=sc, in1=rqe[:, j],
                op0=ALU.mult, op1=ALU.subtract,
            )
        maske = me_pool.tile([P, bs, TE], F16, tag="maske")
        nc.vector.tensor_scalar(
            out=maske, in0=de[:, :, 0], scalar1=0.25, scalar2=None, op0=ALU.is_gt
        )
        nc.vector.scalar_tensor_tensor(
            out=de[:, :, 1], in0=de[:, :, 0], scalar=0.25, in1=maske,
            op0=ALU.add, op1=ALU.subtract,
        )

        # features (a-side first so the coef fold overlaps the e-side Sin)
        fa = fa_pool.tile([P, bs, 2, TD_N], BF16, tag="fa")
        nc.scalar.activation(
            out=fa, in_=da, func=mybir.ActivationFunctionType.Sin, scale=TWO_PI
        )
        ge = ge_pool.tile([P, bs, 2, TE], BF16, tag="ge")
        nc.scalar.activation(
            out=ge, in_=de, func=mybir.ActivationFunctionType.Sin, scale=TWO_PI
        )

        # fold c[d,m] = V_d*b_m into both a-side halves: (fa * V) * b_m
        fsc = fs_pool.tile([P, bs, 2, TD_N], BF16, tag="fsc")
        for j in range(bs):
            nc.vector.tensor_scalar(
                out=fsc[:, j], in0=fa[:, j], scalar1=v_sb,
                scalar2=float(BCOEF[m0 + j]), op0=ALU.mult, op1=ALU.mult,
            )

        # scoreT += G_sin^T F~_cos + G_cos^T F~_sin
        for j in range(bs):
            for half in (0, 1):
                for c in range(N_CHUNK):
                    nc.tensor.matmul(
                        st[:, c, :],
                        ge[:, j, half, c * P : (c + 1) * P],
                        fsc[:, j, 1 - half, :],
                        start=(mm_i == 0),
                        stop=(mm_i == n_mm - 1),
                        skip_group_check=True,
                    )
                mm_i += 1
        m0 += bs

    if DEBUG_SCORE is not None:
        dbg = consts.tile([P, N_CHUNK, TD_N], F32)
        nc.vector.tensor_copy(dbg, st)
        nc.sync.dma_start(out=DEBUG_SCORE[0], in_=dbg)

    # ---- softmax + context, chunk-staggered ----
    # one full PSUM bank per block: the two blocks' accumulation groups
    # interleave, so they must not share a bank
    ctx_ps = [
        ctx_ps_pool.tile([P, 512], F32, tag=f"ctx{b}", name=f"ctx_ps{b}")[:, 0 : P + 1]
        for b in range(2)
    ]
    for c in range(N_CHUNK):
        escT = esc_pool.tile([P, TD_N], BF16, tag=f"escT{c}")
        nc.scalar.activation(
            out=escT, in_=st[:, c, :], func=mybir.ActivationFunctionType.Exp
        )
        for blk in range(2):
            nc.tensor.matmul(
                ctx_ps[blk], escT[:, blk * P : (blk + 1) * P], enc_ones[:, c, :],
                start=(c == 0), stop=(c == N_CHUNK - 1),
            )
    dma_engs = [nc.sync, nc.scalar]
    for blk in range(2):
        recip = out_pool.tile([P, 1], F32, tag=f"recip{blk}")
        nc.vector.reciprocal(recip, ctx_ps[blk][:, P : P + 1])
        ctx_sb = out_pool.tile([P, P], F32, tag=f"ctx_sb{blk}")
        nc.vector.tensor_scalar_mul(out=ctx_sb, in0=ctx_ps[blk][:, 0:P], scalar1=recip)
        dma_engs[blk].dma_start(
            out=out_ap[blk * P : (blk + 1) * P, :], in_=ctx_sb
        )


def build_program(td_n=TD_N):
    nc = bacc.Bacc("TRN2", target_bir_lowering=False, debug=False)
    decT = nc.dram_tensor("decT", [P, td_n], F32, kind="ExternalInput").ap()
    encT = nc.dram_tensor("encT", [P, TE], F16, kind="ExternalInput").ap()
    enc_ones = nc.dram_tensor(
        "enc_ones", [P, N_CHUNK, P + 1], BF16, kind="ExternalInput"
    ).ap()
    v = nc.dram_tensor("v", [P, 1], F32, kind="ExternalInput").ap()
    w = nc.dram_tensor("w", [D, D], F32, kind="ExternalInput").ap()
    out = nc.dram_tensor("ctx_out", [td_n, D], F32, kind="ExternalOutput").ap()
    with tile.TileContext(nc) as tc, ExitStack() as ctx:
        _build_body(ctx, tc, out, decT, encT, enc_ones, v, w)
    nc.compile()
    return nc


def _prep_core_inputs(dec_slice, enc_b, w, v, bf16):
    """Host-side layout marshalling for one core (no reference FLOPs)."""
    decT = np.ascontiguousarray(dec_slice.T)                       # [d, td]
    encT = np.ascontiguousarray(enc_b.T).astype(np.float16)        # [d, te]
    enc_ones = np.ones((P, N_CHUNK, P + 1), dtype=np.float32)
    enc_ones[:, :, :P] = enc_b.reshape(N_CHUNK, P, D).transpose(1, 0, 2)
    return {
        "decT": decT,
        "encT": encT,
        "enc_ones": enc_ones.astype(bf16),
        "v": np.ascontiguousarray(v, dtype=np.float32),
        "w": np.ascontiguousarray(w),
    }


_CACHED_NC = None


def _run(inputs, trace=False):
    global _CACHED_NC
    if _CACHED_NC is None:
        _CACHED_NC = build_program()
    nc = _CACHED_NC
    bf16 = mybir.dt.np(BF16)

    dec = np.ascontiguousarray(inputs["decoder_outputs"], dtype=np.float32)
    enc = np.ascontiguousarray(inputs["encoder_outputs"], dtype=np.float32)
    w = np.ascontiguousarray(inputs["W"], dtype=np.float32)
    v = np.ascontiguousarray(inputs["V"], dtype=np.float32)

    in_maps = []
    for c in range(N_CORES):
        b, h = divmod(c, 2)
        in_maps.append(
            _prep_core_inputs(dec[b, h * TD_N : (h + 1) * TD_N], enc[b], w, v, bf16)
        )
    res = run_bass_kernel_spmd(nc, in_maps, core_ids=list(range(N_CORES)), trace=trace)
    out = np.zeros((B, TD, D), dtype=np.float32)
    for c in range(N_CORES):
        b, h = divmod(c, 2)
        out[b, h * TD_N : (h + 1) * TD_N] = res.results[c]["ctx_out"]
    return out, res


def kernel(**inputs):
    out, _ = _run(inputs, trace=False)
    return out


if __name__ == "__main__":
    rng = np.random.default_rng(0)
    inputs = {
        "decoder_outputs": rng.standard_normal((B, TD, D)).astype(np.float32),
        "encoder_outputs": rng.standard_normal((B, TE, D)).astype(np.float32),
        "W": (rng.uniform(-0.15, 0.15, (D, D))).astype(np.float32),
        "V": (rng.uniform(-0.21, 0.21, (D, 1))).astype(np.float32),
    }
    out = kernel(**inputs)
    print("ran, output shape", out.shape)
